# revision 8
# baseline (speedup 1.0000x reference)
"""Multi-head attention block (B=8, S=1024, D=768, H=12) on 8 TRN2 NeuronCores.

Data-parallel: one batch element per core (attention is independent per batch).
Per-core pipeline (bf16 matmuls, fp32 PSUM accumulation), restructured so the
ACT engine (softmax exp, the 2nd-busiest engine) never starves at head-pair
boundaries: score matmuls, PV matmuls, and QK-projection matmuls interleave at
key-chunk granularity in one PE instruction stream.

  prologue: HAM warm-up matmuls at t=0; x as two contiguous DMAs (sync+scalar
            queues), weights one ~1.2MB DMA each across sync/scalar/gpsimd
            queues; x -> xT via PE transpose; QT/KT chunks 0,1.
  pair 0:   scores+exp for pair 0 interleaved with V = x Wv (+bv), stored as
            [128,12,65] with a ones column per head (rowsum rides in PV).
  pair c:   slot kc: scores+exp(pair c+1, kc) | PV(pair c) 2 chunks | 3 matmuls
            of QK chunk c+2.  Softmax denominators: rowsum row -> K=128
            broadcast matmul -> 1/x -> scale (norm of half 0 at slot 4, half 1
            at pair end).
  tail:     out = O_cat @ Wp (+bp), fp32, DMA halves on both HWDGE queues.

PSUM budget (8 banks): scores 3x[128,512]f32 + ov/V/proj 2x2banks +
aux (qk halves / bcast) 1x1 bank.
"""

import numpy as np

B, S, DIM, H = 8, 1024, 768, 12
HD = DIM // H          # 64
SCALE = HD ** -0.5
N_CORES = 8
KC = DIM // 128        # 6 d-chunks
SC = S // 128          # 8 seq-chunks

_CACHE = {}


def _build():
    import concourse.mybir as mybir
    import concourse.tile as tile
    from concourse import bacc
    from concourse.masks import make_identity

    f32 = mybir.dt.float32
    bf16 = mybir.dt.bfloat16
    EXP = mybir.ActivationFunctionType.Exp

    nc = bacc.Bacc()

    x_ext = nc.declare_dram_parameter("x", [S, DIM], bf16, isOutput=False)
    Wq_ext = nc.declare_dram_parameter("Wq", [DIM, DIM], bf16, isOutput=False)
    bq_ext = nc.declare_dram_parameter("bq", [DIM], f32, isOutput=False)
    Wk_ext = nc.declare_dram_parameter("Wk", [DIM, DIM], bf16, isOutput=False)
    bk_ext = nc.declare_dram_parameter("bk", [DIM], f32, isOutput=False)
    Wv_ext = nc.declare_dram_parameter("Wv", [DIM, DIM], bf16, isOutput=False)
    bv_ext = nc.declare_dram_parameter("bv", [DIM], bf16, isOutput=False)
    Wp_ext = nc.declare_dram_parameter("Wp", [DIM, DIM], bf16, isOutput=False)
    bp_ext = nc.declare_dram_parameter("bp", [DIM], bf16, isOutput=False)
    out_ext = nc.declare_dram_parameter("out", [S, DIM], f32, isOutput=True)

    HALVES = ((0, 512), (512, 1024))
    VHALVES = ((0, 512), (512, DIM))

    with tile.TileContext(nc) as tc:
        with tc.tile_pool(name="persist", bufs=1) as sb, \
             tc.tile_pool(name="ps", bufs=1, space="PSUM") as ps:

            def st_tile():
                return ps.tile([128, 512], f32, tag="st", bufs=3, name="st")

            def ov_tile(shape, dtype=f32, name="ovt"):
                return ps.tile(list(shape), dtype, tag="ov", bufs=2, name=name)

            def aux_tile(shape, dtype=f32, name="aux"):
                return ps.tile(list(shape), dtype, tag="aux", bufs=1, name=name)

            # ---- constants (gpsimd identity first: its queue also carries
            # the SWDGE weight DMA later) ----
            ident = sb.tile([128, 128], bf16)
            make_identity(nc, ident)
            ones2d = sb.tile([128, 128], bf16)
            nc.vector.memset(ones2d, 1.0)
            rs_z = sb.tile([128, S], bf16)
            nc.vector.memset(rs_z, 0.0)
            V12 = [sb.tile([128, H, HD + 1], bf16, name=f"V12_{s8}") for s8 in range(SC)]
            for s8 in range(SC):
                nc.vector.memset(V12[s8][:, :, HD:HD + 1], 1.0)

            # HAM warm-up: dummy matmuls from t=0 while the input DMAs run,
            # so every real matmul sees the 8/8 clock gate.
            for _ in range(14):
                wu = aux_tile([128, 512], name="wu")
                nc.tensor.matmul(wu, ones2d, rs_z[:, 0:512], start=True, stop=True)

            # ---- input DMAs over three queues ----
            x_sb = sb.tile([128, SC, DIM], bf16, name="x_sb")
            nc.sync.dma_start(
                out=x_sb[:, 0:4, :],
                in_=x_ext[0:512, :].rearrange("(a p) n -> p a n", p=128))
            nc.scalar.dma_start(
                out=x_sb[:, 4:8, :],
                in_=x_ext[512:1024, :].rearrange("(a p) n -> p a n", p=128))

            bq_sb = sb.tile([128, KC], f32)
            nc.gpsimd.dma_start(out=bq_sb, in_=bq_ext[:].rearrange("(c p) -> p c", p=128))
            bk_sb = sb.tile([128, KC], f32)
            nc.gpsimd.dma_start(out=bk_sb, in_=bk_ext[:].rearrange("(c p) -> p c", p=128))
            bv_row = sb.tile([1, DIM], bf16)
            nc.gpsimd.dma_start(out=bv_row, in_=bv_ext[:].rearrange("(a d) -> a d", a=1))
            bp_row = sb.tile([1, DIM], bf16)
            nc.gpsimd.dma_start(out=bp_row, in_=bp_ext[:].rearrange("(a d) -> a d", a=1))

            def w_load(W_ext, eng, name):
                w = sb.tile([128, KC, DIM], bf16, name=name)
                eng.dma_start(
                    out=w, in_=W_ext[:].rearrange("(c p) n -> p c n", p=128))
                return [w[:, c, :] for c in range(KC)]

            Wq_sb = w_load(Wq_ext, nc.scalar, "Wq")
            Wk_sb = w_load(Wk_ext, nc.sync, "Wk")
            Wv_sb = w_load(Wv_ext, nc.scalar, "Wv")
            Wp_sb = w_load(Wp_ext, nc.gpsimd, "Wp")

            # x -> xT via PE transpose; PSUM->SBUF casts split DVE/ACT
            xT = [sb.tile([128, S], bf16, name=f"xT{c}") for c in range(KC)]
            for c in range(KC):
                xt_ps = ov_tile((128, S), bf16, name="xt_ps")
                for s8 in range(SC):
                    nc.tensor.transpose(
                        xt_ps[:, s8 * 128:(s8 + 1) * 128],
                        x_sb[:, s8, c * 128:(c + 1) * 128], ident)
                nc.vector.tensor_copy(xT[c][:, 0:512], xt_ps[:, 0:512])
                nc.scalar.copy(xT[c][:, 512:1024], xt_ps[:, 512:1024])

            # persistent activations
            QT = [sb.tile([128, S], bf16, name=f"QT{c}") for c in range(KC)]
            KT = [sb.tile([128, S], bf16, name=f"KT{c}") for c in range(KC)]

            # full-width QK chunk (prologue: chunks 0 and 1)
            def qk_full(W_sb, bias_sb, dst, m):
                q_ps = ov_tile((128, S), name="q_ps")
                for k in range(KC):
                    for n0, n1 in HALVES:
                        nc.tensor.matmul(
                            q_ps[:, n0:n1],
                            W_sb[k][:, m * 128:(m + 1) * 128],
                            xT[k][:, n0:n1],
                            start=(k == 0), stop=(k == KC - 1))
                nc.vector.tensor_scalar_add(dst[m], q_ps, bias_sb[:, m:m + 1])

            qk_full(Wq_sb, bq_sb, QT, 0)
            qk_full(Wk_sb, bk_sb, KT, 0)

            # broadcast bv/bp across partitions (K=128 matmul on the
            # zero-padded carrier; row 0 = bias row)
            bv_bc = sb.tile([128, DIM], f32)
            bp_bc = sb.tile([128, DIM], f32)
            for row, bc in ((bv_row, bv_bc), (bp_row, bp_bc)):
                nc.vector.tensor_copy(rs_z[0:1, 0:DIM], row[0:1, :])
                bc_ps = ov_tile((128, DIM), name="bias_ps")
                for n0, n1 in VHALVES:
                    nc.tensor.matmul(bc_ps[:, n0:n1], ones2d,
                                     rs_z[:, n0:n1], start=True, stop=True)
                nc.scalar.copy(bc, bc_ps)

            qk_full(Wq_sb, bq_sb, QT, 1)
            qk_full(Wk_sb, bk_sb, KT, 1)

            with tc.tile_pool(name="pb", bufs=1) as pb:
                OT = [pb.tile([128, S], bf16, name=f"OT{c}") for c in range(KC)]

                # scores + exp for (pair c, key-chunk kc) -> 4 bf16 P^T tiles
                def st_exp(c, kc, into):
                    for hi, (n0, n1) in enumerate(HALVES):
                        st_e = st_tile()
                        st_o = st_tile()
                        nc.tensor.matmul(
                            st_e,
                            KT[c][0:HD, kc * 128:(kc + 1) * 128],
                            QT[c][0:HD, n0:n1], start=True, stop=True)
                        nc.tensor.matmul(
                            st_o,
                            KT[c][HD:128, kc * 128:(kc + 1) * 128],
                            QT[c][HD:128, n0:n1], start=True, stop=True)
                        p_e = pb.tile([128, 512], bf16, tag=f"pt{kc}e{hi}",
                                      bufs=2, name=f"pt{kc}e{hi}")
                        nc.scalar.activation(p_e, st_e, EXP, scale=SCALE)
                        p_o = pb.tile([128, 512], bf16, tag=f"pt{kc}o{hi}",
                                      bufs=2, name=f"pt{kc}o{hi}")
                        nc.scalar.activation(p_o, st_o, EXP, scale=SCALE)
                        into[(0, kc, hi)] = p_e
                        into[(1, kc, hi)] = p_o

                # V chunk half: natural layout out[seq, d] = x Wv (+bv)
                def emit_v_half(s8, hi):
                    n0, n1 = VHALVES[hi]
                    nh = (n1 - n0) // HD
                    vps = ov_tile((128, n1 - n0), name="v_ps")
                    for k in range(KC):
                        nc.tensor.matmul(
                            vps, xT[k][:, s8 * 128:(s8 + 1) * 128],
                            Wv_sb[k][:, n0:n1],
                            start=(k == 0), stop=(k == KC - 1))
                    nc.vector.tensor_add(
                        V12[s8][:, n0 // HD:n1 // HD, 0:HD],
                        vps[:].rearrange("p (h d) -> p h d", h=nh),
                        bv_bc[:, n0:n1].rearrange("p (h d) -> p h d", h=nh))

                # 2 key-chunks of PV accumulation for head 2c+half
                def pv_two(c, half, pts, ov, kcs):
                    for kc2 in kcs:
                        for hi, (n0, n1) in enumerate(HALVES):
                            nc.tensor.matmul(
                                ov[:, n0:n1],
                                V12[kc2][:, 2 * c + half, :],
                                pts[(half, kc2, hi)],
                                start=(kc2 == 0), stop=(kc2 == SC - 1))

                # rowsum -> K=128 broadcast matmul -> 1/x -> scale into OT
                # (broadcast in [64,512] fp32 halves through the 1-bank aux slot)
                def norm_head(ov, c, half):
                    nc.vector.tensor_copy(rs_z[0:1, :], ov[HD:HD + 1, :])
                    base = half * HD
                    rbc = pb.tile([HD, S], f32, tag="rbc", bufs=2, name="rbc")
                    for n0, n1 in HALVES:
                        bch = aux_tile([HD, 512], name="bch")
                        nc.tensor.matmul(bch, ones2d[:, 0:HD],
                                         rs_z[:, n0:n1], start=True, stop=True)
                        nc.vector.reciprocal_approx_fast(rbc[:, n0:n1], bch)
                        nc.vector.tensor_mul(OT[c][base:base + HD, n0:n1],
                                             ov[0:HD, n0:n1], rbc[:, n0:n1])

                # 3 matmuls of QK chunk m (half = slot//2, part = slot%2)
                qk_state = {}

                def qk_piece(which, m, slot):
                    W_sb, bias_sb, dst = ((Wq_sb, bq_sb, QT) if which == 'q'
                                          else (Wk_sb, bk_sb, KT))
                    half, part = slot // 2, slot % 2
                    n0, n1 = HALVES[half]
                    if part == 0:
                        qk_state[which] = aux_tile([128, 512], name="qk_h")
                    t = qk_state[which]
                    for k in range(3 * part, 3 * part + 3):
                        nc.tensor.matmul(
                            t, W_sb[k][:, m * 128:(m + 1) * 128],
                            xT[k][:, n0:n1],
                            start=(k == 0), stop=(k == KC - 1))
                    if part == 1:
                        nc.vector.tensor_scalar_add(
                            dst[m][:, n0:n1], t, bias_sb[:, m:m + 1])

                # ---- pair 0: scores+exp interleaved with V ----
                pts = {}
                for kc in range(SC):
                    st_exp(0, kc, pts)
                    emit_v_half(kc, 0)
                    emit_v_half(kc, 1)

                # ---- main pair loop: fully interleaved slots ----
                for c in range(KC):
                    nxt = {}
                    ov0 = ov1 = None
                    for kc in range(SC):
                        if c + 1 < KC:
                            st_exp(c + 1, kc, nxt)
                        if kc < 4:
                            if kc == 0:
                                ov0 = ov_tile((HD + 1, S), name="ov0")
                            pv_two(c, 0, pts, ov0, [2 * kc, 2 * kc + 1])
                            if c + 2 < KC:
                                qk_piece('q', c + 2, kc)
                        else:
                            if kc == 4:
                                ov1 = ov_tile((HD + 1, S), name="ov1")
                                norm_head(ov0, c, 0)
                            pv_two(c, 1, pts, ov1, [2 * (kc - 4), 2 * (kc - 4) + 1])
                            if c + 2 < KC:
                                qk_piece('k', c + 2, kc - 4)
                    norm_head(ov1, c, 1)
                    pts = nxt

                # ---- out = O_cat @ Wp + bp ----
                for s8 in range(SC):
                    fin = pb.tile([128, DIM], f32, tag="fin", bufs=2, name="fin")
                    for hi, (n0, n1) in enumerate(VHALVES):
                        fps = ov_tile((128, n1 - n0), name="f_ps")
                        for k in range(KC):
                            nc.tensor.matmul(
                                fps, OT[k][:, s8 * 128:(s8 + 1) * 128],
                                Wp_sb[k][:, n0:n1],
                                start=(k == 0), stop=(k == KC - 1))
                        nc.vector.tensor_add(fin[:, n0:n1], fps, bp_bc[:, n0:n1])
                    e0, e1 = (nc.sync, nc.scalar) if s8 % 2 == 0 else (nc.scalar, nc.sync)
                    e0.dma_start(out=out_ext[s8 * 128:(s8 + 1) * 128, 0:512],
                                 in_=fin[:, 0:512])
                    e1.dma_start(out=out_ext[s8 * 128:(s8 + 1) * 128, 512:DIM],
                                 in_=fin[:, 512:DIM])

    nc.compile()
    return nc


def get_nc():
    if "nc" not in _CACHE:
        _CACHE["nc"] = _build()
    return _CACHE["nc"]


def kernel(x, Wq, bq, Wk, bk, Wv, bv, Wp, bp):
    import ml_dtypes
    from concourse.bass_utils import run_bass_kernel_spmd

    nc = get_nc()
    bfl = ml_dtypes.bfloat16
    x = np.ascontiguousarray(np.asarray(x, np.float32).astype(bfl))
    shared = {
        "Wq": np.ascontiguousarray(np.asarray(Wq, np.float32).astype(bfl)),
        "bq": np.ascontiguousarray(np.asarray(bq, np.float32)),
        "Wk": np.ascontiguousarray(np.asarray(Wk, np.float32).astype(bfl)),
        "bk": np.ascontiguousarray(np.asarray(bk, np.float32)),
        "Wv": np.ascontiguousarray(np.asarray(Wv, np.float32).astype(bfl)),
        "bv": np.ascontiguousarray(np.asarray(bv, np.float32).astype(bfl)),
        "Wp": np.ascontiguousarray(np.asarray(Wp, np.float32).astype(bfl)),
        "bp": np.ascontiguousarray(np.asarray(bp, np.float32).astype(bfl)),
    }
    in_maps = [{"x": x[b], **shared} for b in range(N_CORES)]
    res = run_bass_kernel_spmd(nc, in_maps, core_ids=list(range(N_CORES)))
    return np.stack([res.results[i]["out"] for i in range(N_CORES)], axis=0)
